# revision 4
# baseline (speedup 1.0000x reference)
"""Trainium2 Bass kernel for banded local attention (window=128, S=8192, D=1024).

Math refactor exploited here: the reference computes
    scores[t,s] = q_t . k_s   for s in [t-W, t+W), 0 elsewhere
    attn = softmax(scores)    (the zeros participate: exp(0)=1)
    out  = attn @ x
Since exp(0)=1 outside the band:
    out[t] = (sum_{s in win} (e^{sc}-1) x_s + sum_all x_s) / (S + sum_{s in win}(e^{sc}-1))
and scores factor as  sc(t,s) = x_t M x_s^T + a_t + b_s + c  with
    M = Wq^T Wk, a = x @ (bk @ Wq), b = x @ (bq @ Wk), c = bq.bk.
So the device only computes a 256-wide band of scores plus one rank-D
projection u = x @ M, instead of the dense [S,S] attention.

Sharding: sequence split over 8 cores (1024 queries each), keys haloed by
window_size=128 on each side (zero-padded at the edges). Identical SPMD
program on all cores; per-core data carries the halo/boundary handling.
"""

import numpy as np

import concourse.bass as bass
import concourse.tile as tile
from concourse import bacc, mybir
from concourse import bass_utils

# Problem constants (hardcoded per harness contract).
S = 8192
D = 1024
W = 128
NCORES = 8
SL = S // NCORES          # 1024 queries per core
H = SL + 2 * W            # 1280 key halo per core
NT = SL // 128            # 8 query tiles of 128 per core
NB = SL // 256            # 4 blocks of 256 queries per core
HK = 2 * W + 128          # 384 keys seen by one 128-query tile
KAPPA = 15.0              # exp shift for overflow headroom (mathematically neutral)
EK = float(np.exp(-KAPPA))
ZBIAS = float((S - HK) * EK)

# Use the fast single-pass fp32 matmul mode (float32r, 4x faster than fp32)
# for the large matmuls. Flip to False for full-precision fp32 matmuls.
USE_F32R = False

F32 = mybir.dt.float32


def _r(ap):
    return ap.bitcast(mybir.dt.float32r) if USE_F32R else ap


def _emit(tc, dins, dout):
    nc = tc.nc
    xt_d, xh_d, m_d, augl_d, augr_d, mask_d, c_d = (
        dins["xt"], dins["xh"], dins["m"], dins["augl"], dins["augr"],
        dins["mask"], dins["crows"],
    )
    Exp = mybir.ActivationFunctionType.Exp

    from contextlib import ExitStack
    with ExitStack() as ctx:
        perm = ctx.enter_context(tc.tile_pool(name="perm", bufs=1))
        XT = perm.tile([128, 8, H], F32)      # x^T halo: [d%128, d//128, s]
        XH = perm.tile([128, 10, D], F32)     # x halo rows: [s%128, s//128, d]
        US = perm.tile([128, 8, SL], F32)     # u^T = (x M)^T: [d%128, d//128, t]
        MK = perm.tile([128, 4 * 256], F32)   # banded mask, 4 chunks of [128,256]
        CS = perm.tile([1, NT * D], F32)      # per-tile constant row C_j
        AUGL = perm.tile([2, H], F32)         # [valid; b_halo + c*valid]
        AUGR = perm.tile([2, SL], F32)        # [a_local; ones]
        ONESC = perm.tile([128, 1], F32)
        ONESR = perm.tile([1, 128], F32)
        ZB = perm.tile([128, 1], F32)
        KB = perm.tile([128, 1], F32)

        for k in range(8):
            nc.sync.dma_start(out=XT[:, k, :], in_=xt_d[k * 128:(k + 1) * 128, :])
        for k in range(10):
            nc.sync.dma_start(out=XH[:, k, :], in_=xh_d[k * 128:(k + 1) * 128, :])
        nc.sync.dma_start(out=MK, in_=mask_d)
        nc.sync.dma_start(out=CS, in_=c_d)
        nc.sync.dma_start(out=AUGL, in_=augl_d)
        nc.sync.dma_start(out=AUGR, in_=augr_d)
        nc.vector.memset(ONESC, 1.0)
        nc.vector.memset(ONESR, 1.0)
        nc.vector.memset(ZB, ZBIAS)
        nc.vector.memset(KB, -KAPPA)

        # ---- Phase 1: u^T = M^T-contraction, uT[do, t] = sum_di M[di, do] xT[di, t]
        with tc.tile_pool(name="mpool", bufs=1) as mpool, \
             tc.tile_pool(name="upsum", bufs=4, space="PSUM") as upsum:
            MS = mpool.tile([128, 8, D], F32)  # [di%128, di//128, do]
            for k in range(8):
                nc.sync.dma_start(out=MS[:, k, :], in_=m_d[k * 128:(k + 1) * 128, :])
            for mc in range(8):
                for nn in range(2):
                    pu = upsum.tile([128, 512], F32, tag="pu", name=f"pu_{mc}_{nn}")
                    for kc in range(8):
                        nc.tensor.matmul(
                            pu,
                            _r(MS[:, kc, mc * 128:(mc + 1) * 128]),
                            _r(XT[:, kc, W + nn * 512: W + nn * 512 + 512]),
                            start=(kc == 0), stop=(kc == 7),
                        )
                    nc.any.tensor_copy(US[:, mc, nn * 512: nn * 512 + 512], pu)

        spool = ctx.enter_context(tc.tile_pool(name="spool", bufs=2, space="PSUM"))
        numpool = ctx.enter_context(tc.tile_pool(name="numpool", bufs=3, space="PSUM"))
        zpool = ctx.enter_context(tc.tile_pool(name="zpool", bufs=1, space="PSUM"))
        epool = ctx.enter_context(tc.tile_pool(name="epool", bufs=2))
        opool = ctx.enter_context(tc.tile_pool(name="opool", bufs=2))
        small = ctx.enter_context(tc.tile_pool(name="small", bufs=2))

        # ---- Phase 2: banded scores -> e = exp(mask*(scores) - kappa), per
        # block of 256 queries with a 512-key halo, scores held transposed
        # [key, query] so e feeds the value matmul as stationary directly.
        def emit_scores(b):
            ps = spool.tile([128, 4 * 256], F32, tag="ps", name=f"ps{b}")
            for J in range(4):
                sc = 2 * b + J
                col = slice(J * 256, (J + 1) * 256)
                for kc in range(8):
                    nc.tensor.matmul(
                        ps[:, col],
                        _r(XT[:, kc, sc * 128:(sc + 1) * 128]),
                        _r(US[:, kc, b * 256:(b + 1) * 256]),
                        start=(kc == 0), stop=False,
                    )
                # k=2 augmented contraction adds a_t + (b_s + c) in one matmul
                nc.tensor.matmul(
                    ps[:, col],
                    _r(AUGL[:, sc * 128:(sc + 1) * 128]),
                    _r(AUGR[:, b * 256:(b + 1) * 256]),
                    start=False, stop=True,
                )
            em = epool.tile([128, 1024], F32, tag="em", name=f"em{b}")
            nc.vector.tensor_mul(em, ps, MK)
            ee = epool.tile([128, 1024], F32, tag="ee", name=f"ee{b}")
            nc.scalar.activation(ee, em, Exp, bias=KB)
            return ee

        # ---- Phase 3 per 128-query tile: num = e @ x_halo (+C), Z = rowsum(e),
        # out = num / (Z + zbias)
        def emit_num(b, ti, ee):
            j = 2 * b + ti
            pn0 = numpool.tile([128, 512], F32, tag="pn", name=f"pn0_{j}")
            pn1 = numpool.tile([128, 512], F32, tag="pn", name=f"pn1_{j}")
            pz = zpool.tile([128, 1], F32, tag="pz", name=f"pz_{j}")
            les = []
            for J3 in range(3):
                J = ti + J3
                les.append((ee[:, J * 256 + ti * 128: J * 256 + ti * 128 + 128],
                            2 * b + J))
            for J3, (le, sc) in enumerate(les):
                nc.tensor.matmul(pn0, _r(le), _r(XH[:, sc, 0:512]),
                                 start=(J3 == 0), stop=False)
            nc.tensor.matmul(pn0, _r(ONESR), _r(CS[0:1, j * D: j * D + 512]),
                             start=False, stop=True)
            for J3, (le, sc) in enumerate(les):
                nc.tensor.matmul(pn1, _r(le), _r(XH[:, sc, 512:1024]),
                                 start=(J3 == 0), stop=False)
            nc.tensor.matmul(pn1, _r(ONESR), _r(CS[0:1, j * D + 512: (j + 1) * D]),
                             start=False, stop=True)
            for J3, (le, sc) in enumerate(les):
                nc.tensor.matmul(pz, le, ONESC, start=(J3 == 0), stop=(J3 == 2))

            zt = small.tile([128, 1], F32, tag="zt", name=f"zt{j}")
            nc.vector.tensor_add(zt, pz, ZB)
            rt = small.tile([128, 1], F32, tag="rt", name=f"rt{j}")
            nc.vector.reciprocal(rt, zt)
            ot = opool.tile([128, 1024], F32, tag="ot", name=f"ot{j}")
            nc.vector.tensor_scalar_mul(ot[:, 0:512], pn0, rt)
            nc.vector.tensor_scalar_mul(ot[:, 512:1024], pn1, rt)
            nc.sync.dma_start(out=dout[j * 128:(j + 1) * 128, :], in_=ot)

        ee_cur = emit_scores(0)
        for b in range(NB):
            ee_next = emit_scores(b + 1) if b + 1 < NB else None
            emit_num(b, 0, ee_cur)
            emit_num(b, 1, ee_cur)
            ee_cur = ee_next


def build():
    nc = bacc.Bacc("TRN2", target_bir_lowering=False, debug=False,
                   num_devices=NCORES)
    dins = {
        "xt": nc.dram_tensor("xt", [D, H], F32, kind="ExternalInput").ap(),
        "xh": nc.dram_tensor("xh", [H, D], F32, kind="ExternalInput").ap(),
        "m": nc.dram_tensor("m", [D, D], F32, kind="ExternalInput").ap(),
        "augl": nc.dram_tensor("augl", [2, H], F32, kind="ExternalInput").ap(),
        "augr": nc.dram_tensor("augr", [2, SL], F32, kind="ExternalInput").ap(),
        "mask": nc.dram_tensor("mask", [128, 4 * 256], F32, kind="ExternalInput").ap(),
        "crows": nc.dram_tensor("crows", [1, NT * D], F32, kind="ExternalInput").ap(),
    }
    dout = nc.dram_tensor("out", [SL, D], F32, kind="ExternalOutput").ap()
    with tile.TileContext(nc) as tc:
        _emit(tc, dins, dout)
    nc.compile()
    return nc


def prep_inputs(x, Wq, bq, Wk, bk):
    """Host-side sharding + tiny precomputations. Returns per-core in_maps."""
    xs = np.ascontiguousarray(x[0], dtype=np.float32)          # [S, D]
    M = np.ascontiguousarray(Wq.T.astype(np.float32) @ Wk.astype(np.float32))
    vq = (bk @ Wq).astype(np.float32)                          # [D]
    vk = (bq @ Wk).astype(np.float32)
    c = float(np.dot(bq.astype(np.float64), bk.astype(np.float64)))
    a_full = xs @ vq                                           # [S]
    b_full = xs @ vk
    csum = np.zeros((S + 1, D), np.float64)
    np.cumsum(xs.astype(np.float64), axis=0, out=csum[1:])
    total = csum[S]
    xsT = np.ascontiguousarray(xs.T)                           # [D, S]

    # Shared banded mask, [key_chunk_row, 4*256]: chunk J covers halo keys
    # [128J, 128J+128) vs the block's 256 queries; in-window iff
    # 0 <= (128J + j) - i < 256.
    jj = np.arange(128)[:, None]
    ii = np.arange(256)[None, :]
    mask = np.zeros((128, 4 * 256), np.float32)
    for J in range(4):
        rel = 128 * J + jj - ii
        mask[:, J * 256:(J + 1) * 256] = ((rel >= 0) & (rel < 256)).astype(np.float32)

    in_maps = []
    for ci in range(NCORES):
        t0 = ci * SL
        lo, hi = t0 - W, t0 + SL + W
        vlo, vhi = max(lo, 0), min(hi, S)
        sl_src = slice(vlo, vhi)
        sl_dst = slice(vlo - lo, vhi - lo)

        xt = np.zeros((D, H), np.float32)
        xt[:, sl_dst] = xsT[:, sl_src]
        xh = np.zeros((H, D), np.float32)
        xh[sl_dst] = xs[sl_src]
        valid = np.zeros(H, np.float32)
        valid[sl_dst] = 1.0
        bh = np.zeros(H, np.float32)
        bh[sl_dst] = b_full[sl_src]
        augl = np.stack([valid, bh + np.float32(c) * valid]).astype(np.float32)
        augr = np.stack([a_full[t0:t0 + SL],
                         np.ones(SL, np.float32)]).astype(np.float32)
        crows = np.zeros((1, NT * D), np.float32)
        for j in range(NT):
            ts_ = t0 + 128 * j
            h0, h1 = max(ts_ - W, 0), min(ts_ + 256, S)
            crows[0, j * D:(j + 1) * D] = ((total - (csum[h1] - csum[h0])) * EK
                                           ).astype(np.float32)
        in_maps.append({
            "xt": xt, "xh": xh, "m": M, "augl": np.ascontiguousarray(augl),
            "augr": np.ascontiguousarray(augr), "mask": mask, "crows": crows,
        })
    return in_maps


def kernel(x, Wq, bq, Wk, bk):
    in_maps = prep_inputs(x, Wq, bq, Wk, bk)
    nc = build()
    res = bass_utils.run_bass_kernel_spmd(nc, in_maps,
                                          core_ids=list(range(NCORES)))
    out = np.concatenate([res.results[i]["out"] for i in range(NCORES)], axis=0)
    return out.reshape(1, S, D).astype(np.float32)
